# revision 6
# baseline (speedup 1.0000x reference)
"""Fused LayerNorm + multi-head self-attention + out-projection for TRN2,
sharded over 8 NeuronCores as (batch x head-group): core c -> batch c//4,
heads [4*(c%4), 4*(c%4)+4).

Per-core math (heads sharded, w_qkv column-sharded, w_out row-sharded):
  xn   = LayerNorm(x[b]) (ln_g folded into weights on host, ln_b via bias terms)
  qk_T = (w_qk.T @ xn_T)                  # [512, 2048]  (q rows 0:256, k rows 256:512)
  V    = xn @ w_v (+ ones col)            # [2048, 4*65] token-major, bf16
  per head h: S_T[k,q] = K_h @ Q_h.T ; P = exp(SCALE*S_T) * keep_T
              [O_h.T | rowsum] = [V_h|1].T.T @ P   (ones-col gives softmax denom)
  O_h.T /= rowsum (via reciprocal + ones-matmul broadcast)
  partial = O.T.T @ w_out[rows for these heads]    # [2048, 1024]
Host sums the 4 partials per batch. exp() needs no running-max: |SCALE*S| is
O(10) for unit-variance inputs, and masked entries are multiplied out after exp.
"""

import numpy as np
import ml_dtypes
from contextlib import ExitStack

import concourse.bass as bass
import concourse.tile as tile
from concourse import mybir
from concourse.masks import make_identity
from concourse.bass_utils import run_bass_kernel_spmd
import bir_fix

F32 = mybir.dt.float32
BF16 = mybir.dt.bfloat16
AF = mybir.ActivationFunctionType

B, N, DIM = 2, 2048, 1024
HEADS, DH = 16, 64
HPC = 4                      # heads per core
SCALE = DH ** -0.5
LN_EPS = 1e-5
P = 128
NT = N // P                  # 16 token tiles
KD = DIM // P                # 8 contraction tiles over model dim
NEG = -30000.0               # additive mask value (unused; multiplicative used)


def build_program(ab=()):
    ab = set(ab)
    nc = bass.Bass()
    x_d = nc.dram_tensor("x", [N, DIM], F32, kind="ExternalInput")
    keep_d = nc.dram_tensor("keep", [HPC, N, N], BF16, kind="ExternalInput")
    wqk_d = nc.dram_tensor("wqk", [DIM, 2 * HPC * DH], F32, kind="ExternalInput")
    wv_d = nc.dram_tensor("wv", [DIM, HPC * DH], F32, kind="ExternalInput")
    wo_d = nc.dram_tensor("wo", [HPC * DH, DIM], F32, kind="ExternalInput")
    qkb_d = nc.dram_tensor("qkb", [2 * HPC * DH], F32, kind="ExternalInput")
    vb_d = nc.dram_tensor("vb", [1, HPC * DH], F32, kind="ExternalInput")
    out_d = nc.dram_tensor("out", [N, DIM], F32, kind="ExternalOutput")

    with tile.TileContext(nc) as tc, ExitStack() as ctx:
        persist = ctx.enter_context(tc.tile_pool(name="persist", bufs=1))

        ident = persist.tile([P, P], F32, tag="ident")
        make_identity(nc, ident)
        ones1 = persist.tile([1, P], F32, tag="ones1")
        nc.vector.memset(ones1, 1.0)
        eps_t = persist.tile([P, 1], F32, tag="eps")
        nc.vector.memset(eps_t, LN_EPS)
        zero_t = persist.tile([P, 1], F32, tag="zero")
        nc.vector.memset(zero_t, 0.0)

        # weights
        wqk_sb = persist.tile([P, KD, 512], F32, tag="wqk")
        nc.sync.dma_start(out=wqk_sb, in_=wqk_d.rearrange("(k p) c -> p k c", p=P))
        wv_sb = persist.tile([P, KD, 256], F32, tag="wv")
        nc.sync.dma_start(out=wv_sb, in_=wv_d.rearrange("(k p) c -> p k c", p=P))
        wo_sb = persist.tile([P, 2, DIM], F32, tag="wo")
        nc.sync.dma_start(out=wo_sb, in_=wo_d.rearrange("(k p) c -> p k c", p=P))
        qkb_sb = persist.tile([P, 4], F32, tag="qkb")
        nc.sync.dma_start(out=qkb_sb, in_=qkb_d.rearrange("(t p) -> p t", p=P))
        vb_sb = persist.tile([1, 256], F32, tag="vb")
        nc.sync.dma_start(out=vb_sb, in_=vb_d[:, :])

        # persistent activations
        qkT = persist.tile([P, 4, N], F32, tag="qkT")      # rows: [q01, q23, k01, k23]
        v_all = persist.tile([P, NT, HPC, DH + 1], BF16, tag="v_all")
        nc.gpsimd.memset(v_all[:, :, :, DH:DH + 1], 1.0)
        o_sb = persist.tile([P, 2, N], F32, tag="o_sb")    # O_T rows: [h01, h23]

        # ---------------- Phase 1: LN + transpose + QKV/V matmuls -------------
        with tc.tile_pool(name="xnT_pool", bufs=1) as xnT_pool, \
             tc.tile_pool(name="xin", bufs=3) as xin_pool, \
             tc.tile_pool(name="stats", bufs=6) as st_pool, \
             tc.tile_pool(name="ps_a", bufs=2, space="PSUM") as ps_a, \
             tc.tile_pool(name="ps_qkv", bufs=2, space="PSUM") as ps_qkv, \
             tc.tile_pool(name="ps_v", bufs=2, space="PSUM") as ps_v:

            xnT = xnT_pool.tile([P, KD, N], F32, tag="xnT")

            for tt in range(NT):
                xt = xin_pool.tile([P, DIM], F32, tag="x")
                nc.sync.dma_start(out=xt, in_=x_d[tt * P:(tt + 1) * P, :])
                # stats
                stats = st_pool.tile([P, 2, 6], F32, tag="bn")
                xt2 = xt.rearrange("p (s d) -> p s d", s=2)
                for s in range(2):
                    nc.vector.bn_stats(out=stats[:, s, :], in_=xt2[:, s, :])
                mv = st_pool.tile([P, 2], F32, tag="mv")
                nc.vector.bn_aggr(out=mv, in_=stats)
                std = st_pool.tile([P, 1], F32, tag="std")
                nc.scalar.activation(std, mv[:, 1:2], AF.Sqrt, bias=eps_t)
                rstd = st_pool.tile([P, 1], F32, tag="rstd")
                nc.vector.reciprocal(rstd, std)
                nmr = st_pool.tile([P, 1], F32, tag="nmr")
                nc.vector.tensor_mul(nmr, mv[:, 0:1], rstd)
                nc.vector.tensor_scalar_mul(nmr, nmr, -1.0)
                # xn = rstd*x - mean*rstd   (in place)
                nc.scalar.activation(xt, xt, AF.Identity, bias=nmr, scale=rstd)
                # transpose 8 [128,128] blocks -> xnT[:, k, tt*128:...]
                for k in range(KD):
                    tp = ps_a.tile([P, P], F32, tag="tp")
                    nc.tensor.transpose(tp, xt[:, k * P:(k + 1) * P], ident)
                    nc.vector.tensor_copy(xnT[:, k, tt * P:(tt + 1) * P], tp)

            # QKV (transposed): psum[cols 128, tok 512] += wqk_tile.T @ xnT
            for m in range(4):
                for tb in range(4):
                    pq = ps_qkv.tile([P, 512], F32, tag="pq")
                    for k in range(KD):
                        nc.tensor.matmul(
                            pq, wqk_sb[:, k, m * P:(m + 1) * P],
                            xnT[:, k, tb * 512:(tb + 1) * 512],
                            start=(k == 0), stop=(k == KD - 1))
                    nc.scalar.activation(qkT[:, m, tb * 512:(tb + 1) * 512], pq,
                                         AF.Identity, bias=qkb_sb[:, m:m + 1])

            # V token-major: psum[tok 128, 256] = ones.T@vb + xnT_tile.T @ wv
            for tt in range(NT):
                pv = ps_v.tile([P, 256], F32, tag="pv")
                if "novb" not in ab:
                    nc.tensor.matmul(pv, ones1, vb_sb, start=True, stop=False)
                for k in range(KD):
                    nc.tensor.matmul(
                        pv, xnT[:, k, tt * P:(tt + 1) * P], wv_sb[:, k, :],
                        start=("novb" in ab and k == 0), stop=(k == KD - 1))
                nc.vector.tensor_copy(
                    v_all[:, tt, :, 0:DH],
                    pv.rearrange("p (h d) -> p h d", h=HPC))

        # ---------------- Phase 2: attention per head -------------------------
        with tc.tile_pool(name="keep", bufs=3) as keep_pool, \
             tc.tile_pool(name="pexp", bufs=4) as p_pool, \
             tc.tile_pool(name="rec", bufs=4) as rec_pool, \
             tc.tile_pool(name="bcs", bufs=2) as bcs_pool, \
             tc.tile_pool(name="ps_s", bufs=2, space="PSUM") as ps_s, \
             tc.tile_pool(name="ps_o", bufs=1, space="PSUM") as ps_o:

            for h in range(([] if "noattn" in ab else list(range(HPC))) and HPC):
                qrow = (h % 2) * DH
                qm, km = h // 2, 2 + h // 2
                o_ps = ps_o.tile([DH + 1, N], F32, tag="o")
                for kt in range(NT):
                    kp = keep_pool.tile([P, N], BF16, tag="keep")
                    nc.sync.dma_start(out=kp, in_=keep_d[h, kt * P:(kt + 1) * P, :])
                    for qb in range(2):
                        sp = ps_s.tile([P, 1024], F32, tag="s")
                        for j in range(2):
                            qs = qb * 1024 + j * 512
                            nc.tensor.matmul(
                                sp[:, j * 512:(j + 1) * 512],
                                qkT[qrow:qrow + DH, km, kt * P:(kt + 1) * P],
                                qkT[qrow:qrow + DH, qm, qs:qs + 512],
                                start=True, stop=True)
                        pe = p_pool.tile([P, 1024], BF16, tag="p")
                        nc.scalar.activation(pe, sp, AF.Exp, bias=zero_t, scale=SCALE)
                        nc.vector.tensor_mul(
                            pe, pe, kp[:, qb * 1024:(qb + 1) * 1024])
                        for j in range(2):
                            qs = qb * 1024 + j * 512
                            nc.tensor.matmul(
                                o_ps[:, qs:qs + 512],
                                v_all[:, kt, h, :],
                                pe[:, j * 512:(j + 1) * 512],
                                start=(kt == 0), stop=(kt == NT - 1))
                # normalize + evict into o_sb
                orow = (h % 2) * DH
                om = h // 2
                for qb in range(2):
                    cs = slice(qb * 1024, (qb + 1) * 1024)
                    rec = rec_pool.tile([1, 1024], F32, tag="rec")
                    nc.vector.reciprocal(rec, o_ps[DH:DH + 1, cs])
                    if "nobc" in ab:
                        nc.vector.tensor_copy(o_sb[orow:orow + DH, om, cs], o_ps[0:DH, cs])
                    else:
                        bc = ps_s.tile([DH, 1024], F32, tag="s")
                        for j in range(2):
                            nc.tensor.matmul(
                                bc[:, j * 512:(j + 1) * 512], ones1[:, 0:DH],
                                rec[:, j * 512:(j + 1) * 512], start=True, stop=True)
                        bcs = bcs_pool.tile([DH, 1024], F32, tag="bcs")
                        nc.scalar.activation(bcs, bc, AF.Copy)
                        nc.vector.tensor_mul(
                            o_sb[orow:orow + DH, om, cs], o_ps[0:DH, cs], bcs)

        # ---------------- Phase 3: out projection -----------------------------
        with tc.tile_pool(name="oev", bufs=3) as oev_pool, \
             tc.tile_pool(name="ps_out", bufs=2, space="PSUM") as ps_out:
            for tt in range(NT):
                po = ps_out.tile([P, DIM], F32, tag="po")
                for nn2 in range(2):
                    for k in range(2):
                        nc.tensor.matmul(
                            po[:, nn2 * 512:(nn2 + 1) * 512],
                            o_sb[:, k, tt * P:(tt + 1) * P],
                            wo_sb[:, k, nn2 * 512:(nn2 + 1) * 512],
                            start=(k == 0), stop=(k == 1))
                ot = oev_pool.tile([P, DIM], F32, tag="ot")
                nc.vector.tensor_copy(ot, po)
                nc.sync.dma_start(out=out_d[tt * P:(tt + 1) * P, :], in_=ot)

    return nc


_NC_CACHE = {}


def _get_program():
    if "nc" not in _NC_CACHE:
        _NC_CACHE["nc"] = bir_fix.apply_to(build_program())
    return _NC_CACHE["nc"]


def _shard_inputs(x, attn_mask, ln_g, ln_b, w_qkv, w_out):
    x = np.asarray(x, np.float32)
    attn_mask = np.asarray(attn_mask)
    ln_g = np.asarray(ln_g, np.float32)
    ln_b = np.asarray(ln_b, np.float32)
    w_qkv = np.asarray(w_qkv, np.float32)
    w_out = np.asarray(w_out, np.float32)

    wg = w_qkv * ln_g[:, None]
    in_maps = []
    for c in range(8):
        b, g = c // 4, c % 4
        hs = slice(g * HPC * DH, (g + 1) * HPC * DH)        # inner dims of group
        wq = wg[:, 0 * DIM:1 * DIM][:, hs]                  # [1024, 256]
        wk = wg[:, 1 * DIM:2 * DIM][:, hs]
        wv = wg[:, 2 * DIM:3 * DIM][:, hs]
        wqk = np.concatenate([wq, wk], axis=1)              # [1024, 512]
        bq = ln_b @ w_qkv[:, 0 * DIM:1 * DIM][:, hs]
        bk = ln_b @ w_qkv[:, 1 * DIM:2 * DIM][:, hs]
        bv = (ln_b @ w_qkv[:, 2 * DIM:3 * DIM][:, hs]).reshape(1, -1)
        keep = (~attn_mask[b, g * HPC:(g + 1) * HPC]).transpose(0, 2, 1)
        in_maps.append({
            "x": np.ascontiguousarray(x[b]),
            "keep": np.ascontiguousarray(keep).astype(ml_dtypes.bfloat16),
            "wqk": np.ascontiguousarray(wqk),
            "wv": np.ascontiguousarray(wv),
            "wo": np.ascontiguousarray(w_out[hs, :]),
            "qkb": np.concatenate([bq, bk]).astype(np.float32),
            "vb": bv.astype(np.float32),
        })
    return in_maps


def kernel(x, attn_mask, ln_g, ln_b, w_qkv, w_out):
    nc = _get_program()
    in_maps = _shard_inputs(x, attn_mask, ln_g, ln_b, w_qkv, w_out)
    res = run_bass_kernel_spmd(nc, in_maps, list(range(8)))
    parts = [r["out"] for r in res.results]
    out = np.stack([parts[0] + parts[1] + parts[2] + parts[3],
                    parts[4] + parts[5] + parts[6] + parts[7]])
    return out.astype(np.float32)


# revision 16
# speedup vs baseline: 1.0044x; 1.0044x over previous
"""Fused LayerNorm + multi-head self-attention + out-projection for TRN2,
sharded over 8 NeuronCores as (batch x head-group): core c -> batch c//4,
heads [4*(c%4), 4*(c%4)+4).

Per-core math (heads sharded, w_qkv column-sharded, w_out row-sharded):
  xn   = LayerNorm(x[b]) (ln_g folded into weights on host, ln_b via bias terms)
  qk_T = (w_qk.T @ xn_T)                  # [512, 2048]  (q rows 0:256, k rows 256:512)
  V    = xn @ w_v (+ ones col)            # [2048, 4*65] token-major, bf16
  per head h: S_T[k,q] = K_h @ Q_h.T ; P = exp(SCALE*S_T) * keep_T
              [O_h.T | rowsum] = [V_h|1].T.T @ P   (ones-col gives softmax denom)
  O_h.T /= rowsum (via reciprocal + ones-matmul broadcast)
  partial = O.T.T @ w_out[rows for these heads]    # [2048, 1024]
Host sums the 4 partials per batch. exp() needs no running-max: |SCALE*S| is
O(10) for unit-variance inputs, and masked entries are multiplied out after exp.
"""

import numpy as np
import ml_dtypes
from contextlib import ExitStack

import concourse.bass as bass
import concourse.tile as tile
from concourse import mybir
from concourse.masks import make_identity
from concourse.bass_utils import run_bass_kernel_spmd
import json as _json


def _split_waits(bir_json_bytes, max_waits=1):
    """This walrus build accepts only one sync-wait command per instruction;
    hoist extra Tile-emitted waits onto standalone EventSemaphore ops."""
    m = _json.loads(bir_json_bytes)
    n = 0
    for func in m["functions"]:
        for blk in func["blocks"]:
            out = []
            for inst in blk["instructions"]:
                si = inst.get("sync_info") or {}
                ow = si.get("on_wait") or []
                if len(ow) > max_waits:
                    for w in ow[:-max_waits]:
                        n += 1
                        out.append({
                            "engine": inst["engine"], "ins": [], "outs": [],
                            "name": f"WSPLIT-{n}",
                            "opcode": "EventSemaphore",
                            "sync_info": {"on_update": [], "on_wait": [w]},
                        })
                    si["on_wait"] = ow[-max_waits:]
                out.append(inst)
            blk["instructions"] = out
    return _json.dumps(m).encode()

F32 = mybir.dt.float32
F32R = mybir.dt.float32r


def _r(ap):
    return ap.bitcast(F32R)
BF16 = mybir.dt.bfloat16
AF = mybir.ActivationFunctionType

B, N, DIM = 2, 2048, 1024
HEADS, DH = 16, 64
HPC = 4                      # heads per core
SCALE = DH ** -0.5
LN_EPS = 1e-5
P = 128
NT = N // P                  # 16 token tiles
KD = DIM // P                # 8 contraction tiles over model dim
NEG = -30000.0               # additive mask value (unused; multiplicative used)


def build_program(ab=()):
    ab = set(ab)
    nc = bass.Bass()
    x_d = nc.dram_tensor("x", [N, DIM], F32, kind="ExternalInput")
    keep_d = nc.dram_tensor("keep", [HPC, N, N], BF16, kind="ExternalInput")
    wqk_d = nc.dram_tensor("wqk", [DIM, 2 * HPC * DH], F32, kind="ExternalInput")
    wv_d = nc.dram_tensor("wv", [DIM, HPC * DH], F32, kind="ExternalInput")
    wo_d = nc.dram_tensor("wo", [HPC * DH, DIM], F32, kind="ExternalInput")
    qkb_d = nc.dram_tensor("qkb", [2 * HPC * DH], F32, kind="ExternalInput")
    vb_d = nc.dram_tensor("vb", [1, HPC * DH], F32, kind="ExternalInput")
    out_d = nc.dram_tensor("out", [N, DIM], F32, kind="ExternalOutput")

    with tile.TileContext(nc) as tc, ExitStack() as ctx:
        persist = ctx.enter_context(tc.tile_pool(name="persist", bufs=1))

        ident = persist.tile([P, P], F32, tag="ident")
        make_identity(nc, ident)
        ones1f = persist.tile([1, P], F32, tag="ones1f")
        nc.vector.memset(ones1f, 1.0)
        ones1 = persist.tile([1, P], F32R, tag="ones1")
        nc.vector.tensor_copy(ones1, ones1f)
        eps_t = persist.tile([P, 1], F32, tag="eps")
        nc.vector.memset(eps_t, LN_EPS)
        zero_t = persist.tile([P, 1], F32, tag="zero")
        nc.vector.memset(zero_t, 0.0)

        # weights: DMA f32 staging then round-copy to f32r for the PE
        wqk_st = persist.tile([P, KD, 512], F32, tag="wqk_st")
        nc.sync.dma_start(out=wqk_st, in_=wqk_d.rearrange("(k p) c -> p k c", p=P))
        wqk_sb = persist.tile([P, KD, 512], F32R, tag="wqk")
        nc.vector.tensor_copy(wqk_sb, wqk_st)
        wv_st = persist.tile([P, KD, 256], F32, tag="wv_st")
        nc.sync.dma_start(out=wv_st, in_=wv_d.rearrange("(k p) c -> p k c", p=P))
        wv_sb = persist.tile([P, KD, 256], F32R, tag="wv")
        nc.vector.tensor_copy(wv_sb, wv_st)
        wo_st = persist.tile([P, 2, DIM], F32, tag="wo_st")
        nc.sync.dma_start(out=wo_st, in_=wo_d.rearrange("(k p) c -> p k c", p=P))
        wo_sb = persist.tile([P, 2, DIM], F32R, tag="wo")
        nc.vector.tensor_copy(wo_sb, wo_st)
        qkb_sb = persist.tile([P, 4], F32, tag="qkb")
        nc.sync.dma_start(out=qkb_sb, in_=qkb_d.rearrange("(t p) -> p t", p=P))
        vb_st = persist.tile([1, 256], F32, tag="vb_st")
        nc.sync.dma_start(out=vb_st, in_=vb_d[:, :])
        vb_sb = persist.tile([1, 256], F32R, tag="vb")
        nc.vector.tensor_copy(vb_sb, vb_st)

        # persistent activations
        qkT = persist.tile([P, 4, N], F32R, tag="qkT")
              # rows: [q01, q23, k01, k23]
        v_all = persist.tile([P, NT, HPC, DH + 1], BF16, tag="v_all")
        nc.gpsimd.memset(v_all[:, :, :, DH:DH + 1], 1.0)
        if "nov" in ab:
            nc.gpsimd.memset(v_all[:, :, :, 0:DH], 0.01)
        o_sb = persist.tile([P, 2, N], F32R, tag="o_sb")    # O_T rows: [h01, h23]

        # ---------------- Phase 1: LN + transpose + QKV/V matmuls -------------
        with tc.tile_pool(name="xnT_pool", bufs=1) as xnT_pool, \
             tc.tile_pool(name="xin", bufs=3) as xin_pool, \
             tc.tile_pool(name="stats", bufs=6) as st_pool, \
             tc.tile_pool(name="ps_a", bufs=2, space="PSUM") as ps_a, \
             tc.tile_pool(name="ps_qkv", bufs=2, space="PSUM") as ps_qkv, \
             tc.tile_pool(name="ps_v", bufs=2, space="PSUM") as ps_v:

            xnT = xnT_pool.tile([P, KD, N], F32R, tag="xnT")

            for tt in range(NT):
                xt = xin_pool.tile([P, DIM], F32, tag="x")
                nc.sync.dma_start(out=xt, in_=x_d[tt * P:(tt + 1) * P, :])
                # stats
                stats = st_pool.tile([P, 2, 6], F32, tag="bn")
                xt2 = xt.rearrange("p (s d) -> p s d", s=2)
                for s in range(2):
                    nc.vector.bn_stats(out=stats[:, s, :], in_=xt2[:, s, :])
                mv = st_pool.tile([P, 2], F32, tag="mv")
                nc.vector.bn_aggr(out=mv, in_=stats)
                std = st_pool.tile([P, 1], F32, tag="std")
                nc.scalar.activation(std, mv[:, 1:2], AF.Sqrt, bias=eps_t)
                rstd = st_pool.tile([P, 1], F32, tag="rstd")
                nc.vector.reciprocal(rstd, std)
                nmr = st_pool.tile([P, 1], F32, tag="nmr")
                nc.vector.tensor_mul(nmr, mv[:, 0:1], rstd)
                nc.vector.tensor_scalar_mul(nmr, nmr, -1.0)
                # xn = rstd*x - mean*rstd   (in place)
                nc.vector.tensor_scalar(xt, xt, rstd, nmr,
                                        op0=mybir.AluOpType.mult,
                                        op1=mybir.AluOpType.add)
                # transpose 8 [128,128] blocks -> xnT[:, k, tt*128:...]
                if "notrans" in ab:
                    if tt == 0:
                        nc.gpsimd.memset(xnT, 0.5)
                else:
                    for k in range(KD):
                        tp = ps_a.tile([P, P], F32, tag="tp")
                        nc.tensor.transpose(tp, xt[:, k * P:(k + 1) * P], ident)
                        nc.vector.tensor_copy(xnT[:, k, tt * P:(tt + 1) * P], tp)

            # QKV (transposed): psum[cols 128, tok 512] += wqk_tile.T @ xnT
            if "noqkv" in ab:
                nc.gpsimd.memset(qkT, 0.01)
            for m in range([] and 4 if False else (0 if "noqkv" in ab else 4)):
                for tb in range(4):
                    pq = ps_qkv.tile([P, 512], F32, tag="pq")
                    for k in range(KD):
                        nc.tensor.matmul(
                            pq, wqk_sb[:, k, m * P:(m + 1) * P],
                            xnT[:, k, tb * 512:(tb + 1) * 512],
                            start=(k == 0), stop=(k == KD - 1))
                    nc.vector.tensor_scalar_add(
                        qkT[:, m, tb * 512:(tb + 1) * 512], pq, qkb_sb[:, m:m + 1])

            # V token-major: psum[tok 128, 256] = ones.T@vb + xnT_tile.T @ wv
            for tt in range(0 if "nov" in ab else NT):
                pv = ps_v.tile([P, 256], F32, tag="pv")
                if "novb" not in ab:
                    nc.tensor.matmul(pv, ones1, vb_sb, start=True, stop=False)
                for k in range(KD):
                    nc.tensor.matmul(
                        pv, xnT[:, k, tt * P:(tt + 1) * P], wv_sb[:, k, :],
                        start=("novb" in ab and k == 0), stop=(k == KD - 1))
                nc.vector.tensor_copy(
                    v_all[:, tt, :, 0:DH],
                    pv.rearrange("p (h d) -> p h d", h=HPC))

        # ---------------- Phase 2: attention per head -------------------------
        with tc.tile_pool(name="keep", bufs=3) as keep_pool, \
             tc.tile_pool(name="pexp", bufs=4) as p_pool, \
             tc.tile_pool(name="rec", bufs=4) as rec_pool, \
             tc.tile_pool(name="bcs", bufs=2) as bcs_pool, \
             tc.tile_pool(name="ps_s", bufs=2, space="PSUM") as ps_s, \
             tc.tile_pool(name="ps_o", bufs=2, space="PSUM") as ps_o:

            for h in ([] if "noattn" in ab else range(HPC)):
                qrow = (h % 2) * DH
                qm, km = h // 2, 2 + h // 2
                for qb in range(2):
                    cs = slice(qb * 1024, (qb + 1) * 1024)
                    o_ps = ps_o.tile([DH + 1, 1024], F32, tag="o")
                    for kt in range(NT):
                        kp = keep_pool.tile([P, 1024], BF16, tag="keep")
                        if "nokeepdma" in ab:
                            nc.gpsimd.memset(kp, 1.0)
                        else:
                            nc.sync.dma_start(
                                out=kp, in_=keep_d[h, kt * P:(kt + 1) * P, cs])
                        sp = ps_s.tile([P, 1024], F32, tag="s")
                        for j in range(2):
                            qs = qb * 1024 + j * 512
                            nc.tensor.matmul(
                                sp[:, j * 512:(j + 1) * 512],
                                qkT[qrow:qrow + DH, km, kt * P:(kt + 1) * P],
                                qkT[qrow:qrow + DH, qm, qs:qs + 512],
                                start=True, stop=True)
                        pe = p_pool.tile([P, 1024], BF16, tag="p")
                        if "expdve" in ab:
                            nc.vector.tensor_copy(pe, sp)
                        else:
                            nc.scalar.activation(pe, sp, AF.Exp, bias=zero_t, scale=SCALE)
                        if "nomult" not in ab:
                            eng = nc.gpsimd if ("gpsmult" in ab and kt % 2) else nc.vector
                            eng.tensor_mul(pe, pe, kp)
                        for j in range(2):
                            nc.tensor.matmul(
                                o_ps[:, j * 512:(j + 1) * 512],
                                v_all[:, kt, h, :],
                                pe[:, j * 512:(j + 1) * 512],
                                start=(kt == 0), stop=(kt == NT - 1))
                # normalize + evict into o_sb
                    orow = (h % 2) * DH
                    om = h // 2
                    rec = rec_pool.tile([1, 1024], F32R, tag="rec")
                    with nc.allow_low_precision(reason="f32r rounding for PE broadcast"):
                        nc.vector.reciprocal(rec, o_ps[DH:DH + 1, :])
                    bc = ps_s.tile([DH, 1024], F32, tag="s")
                    for j in range(2):
                        nc.tensor.matmul(
                            bc[:, j * 512:(j + 1) * 512], ones1[:, 0:DH],
                            rec[:, j * 512:(j + 1) * 512], start=True, stop=True)
                    bcs = bcs_pool.tile([DH, 1024], F32, tag="bcs")
                    nc.scalar.activation(bcs, bc, AF.Copy)
                    nc.vector.tensor_mul(
                        o_sb[orow:orow + DH, om, cs], o_ps[0:DH, :], bcs)

        # ---------------- Phase 3: out projection -----------------------------
        with tc.tile_pool(name="oev", bufs=3) as oev_pool, \
             tc.tile_pool(name="ps_out", bufs=2, space="PSUM") as ps_out:
            for tt in range(NT):
                po = ps_out.tile([P, DIM], F32, tag="po")
                for nn2 in range(2):
                    for k in range(2):
                        nc.tensor.matmul(
                            po[:, nn2 * 512:(nn2 + 1) * 512],
                            o_sb[:, k, tt * P:(tt + 1) * P],
                            wo_sb[:, k, nn2 * 512:(nn2 + 1) * 512],
                            start=(k == 0), stop=(k == 1))
                ot = oev_pool.tile([P, DIM], F32, tag="ot")
                nc.vector.tensor_copy(ot, po)
                nc.sync.dma_start(out=out_d[tt * P:(tt + 1) * P, :], in_=ot)

    return nc


_NC_CACHE = {}


def _get_program():
    if "nc" not in _NC_CACHE:
        nc = build_program()
        data = _split_waits(nc.to_json_bytes())
        nc.to_json_bytes = lambda: data
        _NC_CACHE["nc"] = nc
    return _NC_CACHE["nc"]


def _shard_inputs(x, attn_mask, ln_g, ln_b, w_qkv, w_out):
    x = np.asarray(x, np.float32)
    attn_mask = np.asarray(attn_mask)
    ln_g = np.asarray(ln_g, np.float32)
    ln_b = np.asarray(ln_b, np.float32)
    w_qkv = np.asarray(w_qkv, np.float32)
    w_out = np.asarray(w_out, np.float32)

    wg = w_qkv * ln_g[:, None]
    in_maps = []
    for c in range(8):
        b, g = c // 4, c % 4
        hs = slice(g * HPC * DH, (g + 1) * HPC * DH)        # inner dims of group
        wq = wg[:, 0 * DIM:1 * DIM][:, hs]                  # [1024, 256]
        wk = wg[:, 1 * DIM:2 * DIM][:, hs]
        wv = wg[:, 2 * DIM:3 * DIM][:, hs]
        wqk = np.concatenate([wq, wk], axis=1)              # [1024, 512]
        bq = ln_b @ w_qkv[:, 0 * DIM:1 * DIM][:, hs]
        bk = ln_b @ w_qkv[:, 1 * DIM:2 * DIM][:, hs]
        bv = (ln_b @ w_qkv[:, 2 * DIM:3 * DIM][:, hs]).reshape(1, -1)
        keep = (~attn_mask[b, g * HPC:(g + 1) * HPC]).transpose(0, 2, 1)
        in_maps.append({
            "x": np.ascontiguousarray(x[b]),
            "keep": np.ascontiguousarray(keep).astype(ml_dtypes.bfloat16),
            "wqk": np.ascontiguousarray(wqk),
            "wv": np.ascontiguousarray(wv),
            "wo": np.ascontiguousarray(w_out[hs, :]),
            "qkb": np.concatenate([bq, bk]).astype(np.float32),
            "vb": bv.astype(np.float32),
        })
    return in_maps


def kernel(x, attn_mask, ln_g, ln_b, w_qkv, w_out):
    nc = _get_program()
    in_maps = _shard_inputs(x, attn_mask, ln_g, ln_b, w_qkv, w_out)
    res = run_bass_kernel_spmd(nc, in_maps, list(range(8)))
    parts = [r["out"] for r in res.results]
    out = np.stack([parts[0] + parts[1] + parts[2] + parts[3],
                    parts[4] + parts[5] + parts[6] + parts[7]])
    return out.astype(np.float32)


# revision 18
# speedup vs baseline: 19662.5928x; 19576.9753x over previous
"""Fused LayerNorm + multi-head self-attention + out-projection for TRN2,
sharded over 8 NeuronCores as (batch x head-group): core c -> batch c//4,
heads [4*(c%4), 4*(c%4)+4).

Per-core math (heads sharded, w_qkv column-sharded, w_out row-sharded):
  xn   = LayerNorm(x[b]) (ln_g folded into weights on host, ln_b via bias terms)
  qk_T = (w_qk.T @ xn_T)                  # [512, 2048]  (q rows 0:256, k rows 256:512)
  V    = xn @ w_v (+ ones col)            # [2048, 4*65] token-major, bf16
  per head h: S_T[k,q] = K_h @ Q_h.T ; P = exp(SCALE*S_T) * keep_T
              [O_h.T | rowsum] = [V_h|1].T.T @ P   (ones-col gives softmax denom)
  O_h.T /= rowsum (via reciprocal + ones-matmul broadcast)
  partial = O.T.T @ w_out[rows for these heads]    # [2048, 1024]
Host sums the 4 partials per batch. exp() needs no running-max: |SCALE*S| is
O(10) for unit-variance inputs, and masked entries are multiplied out after exp.
"""

import numpy as np
import ml_dtypes
from contextlib import ExitStack

import concourse.bass as bass
import concourse.tile as tile
from concourse import mybir
from concourse.masks import make_identity
from concourse.bass_utils import run_bass_kernel_spmd
import json as _json


def _split_waits(bir_json_bytes, max_waits=1):
    """This walrus build accepts only one sync-wait command per instruction;
    hoist extra Tile-emitted waits onto standalone EventSemaphore ops."""
    m = _json.loads(bir_json_bytes)
    n = 0
    for func in m["functions"]:
        for blk in func["blocks"]:
            out = []
            for inst in blk["instructions"]:
                si = inst.get("sync_info") or {}
                ow = si.get("on_wait") or []
                if len(ow) > max_waits:
                    for w in ow[:-max_waits]:
                        n += 1
                        out.append({
                            "engine": inst["engine"], "ins": [], "outs": [],
                            "name": f"WSPLIT-{n}",
                            "opcode": "EventSemaphore",
                            "sync_info": {"on_update": [], "on_wait": [w]},
                        })
                    si["on_wait"] = ow[-max_waits:]
                out.append(inst)
            blk["instructions"] = out
    return _json.dumps(m).encode()

F32 = mybir.dt.float32
F32R = mybir.dt.float32r


def _r(ap):
    return ap.bitcast(F32R)
BF16 = mybir.dt.bfloat16
AF = mybir.ActivationFunctionType

B, N, DIM = 2, 2048, 1024
HEADS, DH = 16, 64
HPC = 4                      # heads per core
SCALE = DH ** -0.5
LN_EPS = 1e-5
P = 128
NT = N // P                  # 16 token tiles
KD = DIM // P                # 8 contraction tiles over model dim
NEG = -30000.0               # additive mask value (unused; multiplicative used)


def build_program(ab=()):
    ab = set(ab)
    nc = bass.Bass()
    x_d = nc.dram_tensor("x", [N, DIM], F32, kind="ExternalInput")
    keep_d = nc.dram_tensor("keep", [HPC, N, N], BF16, kind="ExternalInput")
    wqk_d = nc.dram_tensor("wqk", [DIM, 2 * HPC * DH], F32, kind="ExternalInput")
    wv_d = nc.dram_tensor("wv", [DIM, HPC * DH], F32, kind="ExternalInput")
    wo_d = nc.dram_tensor("wo", [HPC * DH, DIM], F32, kind="ExternalInput")
    qkb_d = nc.dram_tensor("qkb", [2 * HPC * DH], F32, kind="ExternalInput")
    vb_d = nc.dram_tensor("vb", [1, HPC * DH], F32, kind="ExternalInput")
    out_d = nc.dram_tensor("out", [N, DIM], F32, kind="ExternalOutput")

    with tile.TileContext(nc) as tc, ExitStack() as ctx:
        persist = ctx.enter_context(tc.tile_pool(name="persist", bufs=1))

        ident = persist.tile([P, P], F32, tag="ident")
        make_identity(nc, ident)
        ones1f = persist.tile([1, P], F32, tag="ones1f")
        nc.vector.memset(ones1f, 1.0)
        ones1 = persist.tile([1, P], F32R, tag="ones1")
        nc.vector.tensor_copy(ones1, ones1f)
        eps_t = persist.tile([P, 1], F32, tag="eps")
        nc.vector.memset(eps_t, LN_EPS)
        zero_t = persist.tile([P, 1], F32, tag="zero")
        nc.vector.memset(zero_t, 0.0)

        # weights: DMA f32 staging then round-copy to f32r for the PE
        wqk_st = persist.tile([P, KD, 512], F32, tag="wqk_st")
        nc.sync.dma_start(out=wqk_st, in_=wqk_d.rearrange("(k p) c -> p k c", p=P))
        wqk_sb = persist.tile([P, KD, 512], F32R, tag="wqk")
        nc.vector.tensor_copy(wqk_sb, wqk_st)
        wv_st = persist.tile([P, KD, 256], F32, tag="wv_st")
        nc.sync.dma_start(out=wv_st, in_=wv_d.rearrange("(k p) c -> p k c", p=P))
        wv_sb = persist.tile([P, KD, 256], F32R, tag="wv")
        nc.vector.tensor_copy(wv_sb, wv_st)
        wo_st = persist.tile([P, 2, DIM], F32, tag="wo_st")
        nc.sync.dma_start(out=wo_st, in_=wo_d.rearrange("(k p) c -> p k c", p=P))
        wo_sb = persist.tile([P, 2, DIM], F32R, tag="wo")
        nc.vector.tensor_copy(wo_sb, wo_st)
        qkb_sb = persist.tile([P, 4], F32, tag="qkb")
        nc.sync.dma_start(out=qkb_sb, in_=qkb_d.rearrange("(t p) -> p t", p=P))
        vb_st = persist.tile([1, 256], F32, tag="vb_st")
        nc.sync.dma_start(out=vb_st, in_=vb_d[:, :])
        vb_sb = persist.tile([1, 256], F32R, tag="vb")
        nc.vector.tensor_copy(vb_sb, vb_st)

        # persistent activations
        qkT = persist.tile([P, 4, N], F32R, tag="qkT")
              # rows: [q01, q23, k01, k23]
        v_all = persist.tile([P, NT, HPC, DH + 1], BF16, tag="v_all")
        nc.gpsimd.memset(v_all[:, :, :, DH:DH + 1], 1.0)
        if "nov" in ab:
            nc.gpsimd.memset(v_all[:, :, :, 0:DH], 0.01)
        o_sb = persist.tile([P, 2, N], F32R, tag="o_sb")    # O_T rows: [h01, h23]

        # ---------------- Phase 1: LN + transpose + QKV/V matmuls -------------
        with tc.tile_pool(name="xnT_pool", bufs=1) as xnT_pool, \
             tc.tile_pool(name="xin", bufs=3) as xin_pool, \
             tc.tile_pool(name="stats", bufs=6) as st_pool, \
             tc.tile_pool(name="ps_a", bufs=4, space="PSUM") as ps_a, \
             tc.tile_pool(name="ps_qkv", bufs=2, space="PSUM") as ps_qkv, \
             tc.tile_pool(name="ps_v", bufs=2, space="PSUM") as ps_v:

            xnT = xnT_pool.tile([P, KD, N], F32R, tag="xnT")

            for tt in range(NT):
                xt = xin_pool.tile([P, DIM], F32, tag="x")
                nc.sync.dma_start(out=xt, in_=x_d[tt * P:(tt + 1) * P, :])
                # stats
                stats = st_pool.tile([P, 2, 6], F32, tag="bn")
                xt2 = xt.rearrange("p (s d) -> p s d", s=2)
                for s in range(2):
                    nc.vector.bn_stats(out=stats[:, s, :], in_=xt2[:, s, :])
                mv = st_pool.tile([P, 2], F32, tag="mv")
                nc.vector.bn_aggr(out=mv, in_=stats)
                std = st_pool.tile([P, 1], F32, tag="std")
                nc.scalar.activation(std, mv[:, 1:2], AF.Sqrt, bias=eps_t)
                rstd = st_pool.tile([P, 1], F32, tag="rstd")
                nc.vector.reciprocal(rstd, std)
                nmr = st_pool.tile([P, 1], F32, tag="nmr")
                nc.vector.tensor_mul(nmr, mv[:, 0:1], rstd)
                nc.vector.tensor_scalar_mul(nmr, nmr, -1.0)
                # xn = rstd*x - mean*rstd   (in place)
                nc.vector.tensor_scalar(xt, xt, rstd, nmr,
                                        op0=mybir.AluOpType.mult,
                                        op1=mybir.AluOpType.add)
                # transpose 8 [128,128] blocks -> xnT[:, k, tt*128:...]
                if "notrans" in ab:
                    if tt == 0:
                        nc.gpsimd.memset(xnT, 0.5)
                else:
                    for k in range(KD):
                        tp = ps_a.tile([P, P], F32, tag="tp")
                        nc.tensor.transpose(tp, xt[:, k * P:(k + 1) * P], ident)
                        nc.vector.tensor_copy(xnT[:, k, tt * P:(tt + 1) * P], tp)

            # QKV (transposed): psum[cols 128, tok 512] += wqk_tile.T @ xnT
            if "noqkv" in ab:
                nc.gpsimd.memset(qkT, 0.01)
            for m in range([] and 4 if False else (0 if "noqkv" in ab else 4)):
                for tb in range(4):
                    pq = ps_qkv.tile([P, 512], F32, tag="pq")
                    for k in range(KD):
                        nc.tensor.matmul(
                            pq, wqk_sb[:, k, m * P:(m + 1) * P],
                            xnT[:, k, tb * 512:(tb + 1) * 512],
                            start=(k == 0), stop=(k == KD - 1))
                    nc.vector.tensor_scalar_add(
                        qkT[:, m, tb * 512:(tb + 1) * 512], pq, qkb_sb[:, m:m + 1])

            # V token-major: psum[tok 128, 256] = ones.T@vb + xnT_tile.T @ wv
            for tt in range(0 if "nov" in ab else NT):
                pv = ps_v.tile([P, 256], F32, tag="pv")
                if "novb" not in ab:
                    nc.tensor.matmul(pv, ones1, vb_sb, start=True, stop=False)
                for k in range(KD):
                    nc.tensor.matmul(
                        pv, xnT[:, k, tt * P:(tt + 1) * P], wv_sb[:, k, :],
                        start=("novb" in ab and k == 0), stop=(k == KD - 1))
                nc.vector.tensor_copy(
                    v_all[:, tt, :, 0:DH],
                    pv.rearrange("p (h d) -> p h d", h=HPC))

        # ---------------- Phase 2: attention per head -------------------------
        with tc.tile_pool(name="keep", bufs=5) as keep_pool, \
             tc.tile_pool(name="pexp", bufs=6) as p_pool, \
             tc.tile_pool(name="rec", bufs=4) as rec_pool, \
             tc.tile_pool(name="bcs", bufs=2) as bcs_pool, \
             tc.tile_pool(name="ps_s", bufs=2, space="PSUM") as ps_s, \
             tc.tile_pool(name="ps_o", bufs=2, space="PSUM") as ps_o:

            for h in ([] if "noattn" in ab else range(HPC)):
                qrow = (h % 2) * DH
                qm, km = h // 2, 2 + h // 2
                for qb in range(2):
                    cs = slice(qb * 1024, (qb + 1) * 1024)
                    o_ps = ps_o.tile([DH + 1, 1024], F32, tag="o")
                    for kt in range(NT):
                        kp = keep_pool.tile([P, 1024], BF16, tag="keep")
                        if "nokeepdma" in ab:
                            nc.gpsimd.memset(kp, 1.0)
                        else:
                            nc.sync.dma_start(
                                out=kp, in_=keep_d[h, kt * P:(kt + 1) * P, cs])
                        sp = ps_s.tile([P, 1024], F32, tag="s")
                        for j in range(2):
                            qs = qb * 1024 + j * 512
                            nc.tensor.matmul(
                                sp[:, j * 512:(j + 1) * 512],
                                qkT[qrow:qrow + DH, km, kt * P:(kt + 1) * P],
                                qkT[qrow:qrow + DH, qm, qs:qs + 512],
                                start=True, stop=True)
                        pe = p_pool.tile([P, 1024], BF16, tag="p")
                        if "expdve" in ab:
                            nc.vector.tensor_copy(pe, sp)
                        else:
                            nc.scalar.activation(pe, sp, AF.Exp, bias=zero_t, scale=SCALE)
                        if "nomult" not in ab:
                            eng = nc.gpsimd if ("gpsmult" in ab and kt % 2) else nc.vector
                            eng.tensor_mul(pe, pe, kp)
                        for j in range(2):
                            nc.tensor.matmul(
                                o_ps[:, j * 512:(j + 1) * 512],
                                v_all[:, kt, h, :],
                                pe[:, j * 512:(j + 1) * 512],
                                start=(kt == 0), stop=(kt == NT - 1))
                # normalize + evict into o_sb
                    orow = (h % 2) * DH
                    om = h // 2
                    rec = rec_pool.tile([1, 1024], F32R, tag="rec")
                    with nc.allow_low_precision(reason="f32r rounding for PE broadcast"):
                        nc.vector.reciprocal(rec, o_ps[DH:DH + 1, :])
                    bc = ps_s.tile([DH, 1024], F32, tag="s")
                    for j in range(2):
                        nc.tensor.matmul(
                            bc[:, j * 512:(j + 1) * 512], ones1[:, 0:DH],
                            rec[:, j * 512:(j + 1) * 512], start=True, stop=True)
                    bcs = bcs_pool.tile([DH, 1024], F32, tag="bcs")
                    nc.scalar.activation(bcs, bc, AF.Copy)
                    nc.vector.tensor_mul(
                        o_sb[orow:orow + DH, om, cs], o_ps[0:DH, :], bcs)

        # ---------------- Phase 3: out projection -----------------------------
        with tc.tile_pool(name="oev", bufs=3) as oev_pool, \
             tc.tile_pool(name="ps_out", bufs=2, space="PSUM") as ps_out:
            for tt in range(NT):
                po = ps_out.tile([P, DIM], F32, tag="po")
                for nn2 in range(2):
                    for k in range(2):
                        nc.tensor.matmul(
                            po[:, nn2 * 512:(nn2 + 1) * 512],
                            o_sb[:, k, tt * P:(tt + 1) * P],
                            wo_sb[:, k, nn2 * 512:(nn2 + 1) * 512],
                            start=(k == 0), stop=(k == 1))
                ot = oev_pool.tile([P, DIM], F32, tag="ot")
                nc.vector.tensor_copy(ot, po)
                nc.sync.dma_start(out=out_d[tt * P:(tt + 1) * P, :], in_=ot)

    return nc


_NC_CACHE = {}


def _get_program():
    if "nc" not in _NC_CACHE:
        nc = build_program()
        data = _split_waits(nc.to_json_bytes())
        nc.to_json_bytes = lambda: data
        _NC_CACHE["nc"] = nc
    return _NC_CACHE["nc"]


def _shard_inputs(x, attn_mask, ln_g, ln_b, w_qkv, w_out):
    x = np.asarray(x, np.float32)
    attn_mask = np.asarray(attn_mask)
    ln_g = np.asarray(ln_g, np.float32)
    ln_b = np.asarray(ln_b, np.float32)
    w_qkv = np.asarray(w_qkv, np.float32)
    w_out = np.asarray(w_out, np.float32)

    wg = w_qkv * ln_g[:, None]
    in_maps = []
    for c in range(8):
        b, g = c // 4, c % 4
        hs = slice(g * HPC * DH, (g + 1) * HPC * DH)        # inner dims of group
        wq = wg[:, 0 * DIM:1 * DIM][:, hs]                  # [1024, 256]
        wk = wg[:, 1 * DIM:2 * DIM][:, hs]
        wv = wg[:, 2 * DIM:3 * DIM][:, hs]
        wqk = np.concatenate([wq, wk], axis=1)              # [1024, 512]
        bq = ln_b @ w_qkv[:, 0 * DIM:1 * DIM][:, hs]
        bk = ln_b @ w_qkv[:, 1 * DIM:2 * DIM][:, hs]
        bv = (ln_b @ w_qkv[:, 2 * DIM:3 * DIM][:, hs]).reshape(1, -1)
        keep = (~attn_mask[b, g * HPC:(g + 1) * HPC]).transpose(0, 2, 1)
        in_maps.append({
            "x": np.ascontiguousarray(x[b]),
            "keep": np.ascontiguousarray(keep).astype(ml_dtypes.bfloat16),
            "wqk": np.ascontiguousarray(wqk),
            "wv": np.ascontiguousarray(wv),
            "wo": np.ascontiguousarray(w_out[hs, :]),
            "qkb": np.concatenate([bq, bk]).astype(np.float32),
            "vb": bv.astype(np.float32),
        })
    return in_maps


def kernel(x, attn_mask, ln_g, ln_b, w_qkv, w_out):
    nc = _get_program()
    in_maps = _shard_inputs(x, attn_mask, ln_g, ln_b, w_qkv, w_out)
    res = run_bass_kernel_spmd(nc, in_maps, list(range(8)))
    parts = [r["out"] for r in res.results]
    out = np.stack([parts[0] + parts[1] + parts[2] + parts[3],
                    parts[4] + parts[5] + parts[6] + parts[7]])
    return out.astype(np.float32)


# revision 19
# speedup vs baseline: 19774.7061x; 1.0057x over previous
"""Fused LayerNorm + multi-head self-attention + out-projection for TRN2,
sharded over 8 NeuronCores as (batch x head-group): core c -> batch c//4,
heads [4*(c%4), 4*(c%4)+4).

Per-core math (heads sharded, w_qkv column-sharded, w_out row-sharded):
  xn   = LayerNorm(x[b]) (ln_g folded into weights on host, ln_b via bias terms)
  qk_T = (w_qk.T @ xn_T)                  # [512, 2048]  (q rows 0:256, k rows 256:512)
  V    = xn @ w_v (+ ones col)            # [2048, 4*65] token-major, bf16
  per head h: S_T[k,q] = K_h @ Q_h.T ; P = exp(SCALE*S_T) * keep_T
              [O_h.T | rowsum] = [V_h|1].T.T @ P   (ones-col gives softmax denom)
  O_h.T /= rowsum (via reciprocal + ones-matmul broadcast)
  partial = O.T.T @ w_out[rows for these heads]    # [2048, 1024]
Host sums the 4 partials per batch. exp() needs no running-max: |SCALE*S| is
O(10) for unit-variance inputs, and masked entries are multiplied out after exp.
"""

import numpy as np
import ml_dtypes
from contextlib import ExitStack

import concourse.bass as bass
import concourse.tile as tile
from concourse import mybir
from concourse.masks import make_identity
from concourse.bass_utils import run_bass_kernel_spmd
import json as _json


def _split_waits(bir_json_bytes, max_waits=1):
    """This walrus build accepts only one sync-wait command per instruction;
    hoist extra Tile-emitted waits onto standalone EventSemaphore ops."""
    m = _json.loads(bir_json_bytes)
    n = 0
    for func in m["functions"]:
        for blk in func["blocks"]:
            out = []
            for inst in blk["instructions"]:
                si = inst.get("sync_info") or {}
                ow = si.get("on_wait") or []
                if len(ow) > max_waits:
                    for w in ow[:-max_waits]:
                        n += 1
                        out.append({
                            "engine": inst["engine"], "ins": [], "outs": [],
                            "name": f"WSPLIT-{n}",
                            "opcode": "EventSemaphore",
                            "sync_info": {"on_update": [], "on_wait": [w]},
                        })
                    si["on_wait"] = ow[-max_waits:]
                out.append(inst)
            blk["instructions"] = out
    return _json.dumps(m).encode()

F32 = mybir.dt.float32
F32R = mybir.dt.float32r


def _r(ap):
    return ap.bitcast(F32R)
BF16 = mybir.dt.bfloat16
AF = mybir.ActivationFunctionType

B, N, DIM = 2, 2048, 1024
HEADS, DH = 16, 64
HPC = 4                      # heads per core
SCALE = DH ** -0.5
LN_EPS = 1e-5
P = 128
NT = N // P                  # 16 token tiles
KD = DIM // P                # 8 contraction tiles over model dim
NEG = -30000.0               # additive mask value (unused; multiplicative used)


def build_program(ab=()):
    ab = set(ab)
    nc = bass.Bass()
    x_d = nc.dram_tensor("x", [N, DIM], F32, kind="ExternalInput")
    keep_d = nc.dram_tensor("keep", [HPC, N, N], BF16, kind="ExternalInput")
    wqk_d = nc.dram_tensor("wqk", [DIM, 2 * HPC * DH], F32, kind="ExternalInput")
    wv_d = nc.dram_tensor("wv", [DIM, HPC * DH], F32, kind="ExternalInput")
    wo_d = nc.dram_tensor("wo", [HPC * DH, DIM], F32, kind="ExternalInput")
    qkb_d = nc.dram_tensor("qkb", [2 * HPC * DH], F32, kind="ExternalInput")
    vb_d = nc.dram_tensor("vb", [1, HPC * DH], F32, kind="ExternalInput")
    out_d = nc.dram_tensor("out", [N, DIM], F32, kind="ExternalOutput")

    with tile.TileContext(nc) as tc, ExitStack() as ctx:
        persist = ctx.enter_context(tc.tile_pool(name="persist", bufs=1))

        ident = persist.tile([P, P], F32, tag="ident")
        make_identity(nc, ident)
        ones1f = persist.tile([1, P], F32, tag="ones1f")
        nc.vector.memset(ones1f, 1.0)
        ones1 = persist.tile([1, P], F32R, tag="ones1")
        nc.vector.tensor_copy(ones1, ones1f)
        eps_t = persist.tile([P, 1], F32, tag="eps")
        nc.vector.memset(eps_t, LN_EPS)
        zero_t = persist.tile([P, 1], F32, tag="zero")
        nc.vector.memset(zero_t, 0.0)

        # weights: DMA f32 staging then round-copy to f32r for the PE
        wqk_st = persist.tile([P, KD, 512], F32, tag="wqk_st")
        nc.sync.dma_start(out=wqk_st, in_=wqk_d.rearrange("(k p) c -> p k c", p=P))
        wqk_sb = persist.tile([P, KD, 512], F32R, tag="wqk")
        nc.vector.tensor_copy(wqk_sb, wqk_st)
        wv_st = persist.tile([P, KD, 256], F32, tag="wv_st")
        nc.sync.dma_start(out=wv_st, in_=wv_d.rearrange("(k p) c -> p k c", p=P))
        wv_sb = persist.tile([P, KD, 256], F32R, tag="wv")
        nc.vector.tensor_copy(wv_sb, wv_st)
        wo_st = persist.tile([P, 2, DIM], F32, tag="wo_st")
        nc.sync.dma_start(out=wo_st, in_=wo_d.rearrange("(k p) c -> p k c", p=P))
        wo_sb = persist.tile([P, 2, DIM], F32R, tag="wo")
        nc.vector.tensor_copy(wo_sb, wo_st)
        qkb_sb = persist.tile([P, 4], F32, tag="qkb")
        nc.sync.dma_start(out=qkb_sb, in_=qkb_d.rearrange("(t p) -> p t", p=P))
        vb_st = persist.tile([1, 256], F32, tag="vb_st")
        nc.sync.dma_start(out=vb_st, in_=vb_d[:, :])
        vb_sb = persist.tile([1, 256], F32R, tag="vb")
        nc.vector.tensor_copy(vb_sb, vb_st)

        # persistent activations
        qkT = persist.tile([P, 4, N], F32R, tag="qkT")
              # rows: [q01, q23, k01, k23]
        v_all = persist.tile([P, NT, HPC, DH + 1], BF16, tag="v_all")
        nc.gpsimd.memset(v_all[:, :, :, DH:DH + 1], 1.0)
        if "nov" in ab:
            nc.gpsimd.memset(v_all[:, :, :, 0:DH], 0.01)
        o_sb = persist.tile([P, 2, N], F32R, tag="o_sb")    # O_T rows: [h01, h23]

        # ---------------- Phase 1: LN + transpose + QKV/V matmuls -------------
        with tc.tile_pool(name="xnT_pool", bufs=1) as xnT_pool, \
             tc.tile_pool(name="xin", bufs=4) as xin_pool, \
             tc.tile_pool(name="stats", bufs=6) as st_pool, \
             tc.tile_pool(name="ps_a", bufs=4, space="PSUM") as ps_a, \
             tc.tile_pool(name="ps_qkv", bufs=2, space="PSUM") as ps_qkv, \
             tc.tile_pool(name="ps_v", bufs=2, space="PSUM") as ps_v:

            xnT = xnT_pool.tile([P, KD, N], F32R, tag="xnT")

            for tt in range(NT):
                xt = xin_pool.tile([P, DIM], F32, tag="x")
                nc.sync.dma_start(out=xt, in_=x_d[tt * P:(tt + 1) * P, :])
                # stats
                stats = st_pool.tile([P, 2, 6], F32, tag="bn")
                xt2 = xt.rearrange("p (s d) -> p s d", s=2)
                for s in range(2):
                    nc.vector.bn_stats(out=stats[:, s, :], in_=xt2[:, s, :])
                mv = st_pool.tile([P, 2], F32, tag="mv")
                nc.vector.bn_aggr(out=mv, in_=stats)
                std = st_pool.tile([P, 1], F32, tag="std")
                nc.scalar.activation(std, mv[:, 1:2], AF.Sqrt, bias=eps_t)
                rstd = st_pool.tile([P, 1], F32, tag="rstd")
                nc.vector.reciprocal(rstd, std)
                nmr = st_pool.tile([P, 1], F32, tag="nmr")
                nc.vector.tensor_mul(nmr, mv[:, 0:1], rstd)
                nc.vector.tensor_scalar_mul(nmr, nmr, -1.0)
                # xn = rstd*x - mean*rstd   (in place)
                nc.vector.tensor_scalar(xt, xt, rstd, nmr,
                                        op0=mybir.AluOpType.mult,
                                        op1=mybir.AluOpType.add)
                # transpose 8 [128,128] blocks -> xnT[:, k, tt*128:...]
                if "notrans" in ab:
                    if tt == 0:
                        nc.gpsimd.memset(xnT, 0.5)
                else:
                    for k in range(KD):
                        tp = ps_a.tile([P, P], F32, tag="tp")
                        nc.tensor.transpose(tp, xt[:, k * P:(k + 1) * P], ident)
                        nc.vector.tensor_copy(xnT[:, k, tt * P:(tt + 1) * P], tp)

            # QKV (transposed): psum[cols 128, tok 512] += wqk_tile.T @ xnT
            if "noqkv" in ab:
                nc.gpsimd.memset(qkT, 0.01)
            for m in range([] and 4 if False else (0 if "noqkv" in ab else 4)):
                for tb in range(4):
                    pq = ps_qkv.tile([P, 512], F32, tag="pq")
                    for k in range(KD):
                        nc.tensor.matmul(
                            pq, wqk_sb[:, k, m * P:(m + 1) * P],
                            xnT[:, k, tb * 512:(tb + 1) * 512],
                            start=(k == 0), stop=(k == KD - 1))
                    nc.vector.tensor_scalar_add(
                        qkT[:, m, tb * 512:(tb + 1) * 512], pq, qkb_sb[:, m:m + 1])

            # V token-major: psum[tok 128, 256] = ones.T@vb + xnT_tile.T @ wv
            for tt in range(0 if "nov" in ab else NT):
                pv = ps_v.tile([P, 256], F32, tag="pv")
                if "novb" not in ab:
                    nc.tensor.matmul(pv, ones1, vb_sb, start=True, stop=False)
                for k in range(KD):
                    nc.tensor.matmul(
                        pv, xnT[:, k, tt * P:(tt + 1) * P], wv_sb[:, k, :],
                        start=("novb" in ab and k == 0), stop=(k == KD - 1))
                nc.vector.tensor_copy(
                    v_all[:, tt, :, 0:DH],
                    pv.rearrange("p (h d) -> p h d", h=HPC))

        # ---------------- Phase 2: attention per head -------------------------
        with tc.tile_pool(name="keep", bufs=5) as keep_pool, \
             tc.tile_pool(name="pexp", bufs=6) as p_pool, \
             tc.tile_pool(name="rec", bufs=4) as rec_pool, \
             tc.tile_pool(name="bcs", bufs=2) as bcs_pool, \
             tc.tile_pool(name="ps_s", bufs=2, space="PSUM") as ps_s, \
             tc.tile_pool(name="ps_o", bufs=2, space="PSUM") as ps_o:

            for h in ([] if "noattn" in ab else range(HPC)):
                qrow = (h % 2) * DH
                qm, km = h // 2, 2 + h // 2
                for qb in range(2):
                    cs = slice(qb * 1024, (qb + 1) * 1024)
                    o_ps = ps_o.tile([DH + 1, 1024], F32, tag="o")
                    for kt in range(NT):
                        kp = keep_pool.tile([P, 1024], BF16, tag="keep")
                        if "nokeepdma" in ab:
                            nc.gpsimd.memset(kp, 1.0)
                        else:
                            nc.sync.dma_start(
                                out=kp, in_=keep_d[h, kt * P:(kt + 1) * P, cs])
                        sp = ps_s.tile([P, 1024], F32, tag="s")
                        for j in range(2):
                            qs = qb * 1024 + j * 512
                            nc.tensor.matmul(
                                sp[:, j * 512:(j + 1) * 512],
                                qkT[qrow:qrow + DH, km, kt * P:(kt + 1) * P],
                                qkT[qrow:qrow + DH, qm, qs:qs + 512],
                                start=True, stop=True)
                        pe = p_pool.tile([P, 1024], BF16, tag="p")
                        if "expdve" in ab:
                            nc.vector.tensor_copy(pe, sp)
                        else:
                            nc.scalar.activation(pe, sp, AF.Exp, bias=zero_t, scale=SCALE)
                        if "nomult" not in ab:
                            eng = nc.gpsimd if ("gpsmult" in ab and kt % 2) else nc.vector
                            eng.tensor_mul(pe, pe, kp)
                        for j in range(2):
                            nc.tensor.matmul(
                                o_ps[:, j * 512:(j + 1) * 512],
                                v_all[:, kt, h, :],
                                pe[:, j * 512:(j + 1) * 512],
                                start=(kt == 0), stop=(kt == NT - 1))
                # normalize + evict into o_sb
                    orow = (h % 2) * DH
                    om = h // 2
                    rec = rec_pool.tile([1, 1024], F32R, tag="rec")
                    with nc.allow_low_precision(reason="f32r rounding for PE broadcast"):
                        nc.vector.reciprocal(rec, o_ps[DH:DH + 1, :])
                    bc = ps_s.tile([DH, 1024], F32, tag="s")
                    for j in range(2):
                        nc.tensor.matmul(
                            bc[:, j * 512:(j + 1) * 512], ones1[:, 0:DH],
                            rec[:, j * 512:(j + 1) * 512], start=True, stop=True)
                    bcs = bcs_pool.tile([DH, 1024], F32, tag="bcs")
                    nc.vector.tensor_copy(bcs, bc)
                    nc.vector.tensor_mul(
                        o_sb[orow:orow + DH, om, cs], o_ps[0:DH, :], bcs)

        # ---------------- Phase 3: out projection -----------------------------
        with tc.tile_pool(name="oev", bufs=4) as oev_pool, \
             tc.tile_pool(name="ps_out", bufs=2, space="PSUM") as ps_out:
            for tt in range(NT):
                po = ps_out.tile([P, DIM], F32, tag="po")
                for nn2 in range(2):
                    for k in range(2):
                        nc.tensor.matmul(
                            po[:, nn2 * 512:(nn2 + 1) * 512],
                            o_sb[:, k, tt * P:(tt + 1) * P],
                            wo_sb[:, k, nn2 * 512:(nn2 + 1) * 512],
                            start=(k == 0), stop=(k == 1))
                ot = oev_pool.tile([P, DIM], F32, tag="ot")
                nc.vector.tensor_copy(ot, po)
                nc.sync.dma_start(out=out_d[tt * P:(tt + 1) * P, :], in_=ot)

    return nc


_NC_CACHE = {}


def _get_program():
    if "nc" not in _NC_CACHE:
        nc = build_program()
        data = _split_waits(nc.to_json_bytes())
        nc.to_json_bytes = lambda: data
        _NC_CACHE["nc"] = nc
    return _NC_CACHE["nc"]


def _shard_inputs(x, attn_mask, ln_g, ln_b, w_qkv, w_out):
    x = np.asarray(x, np.float32)
    attn_mask = np.asarray(attn_mask)
    ln_g = np.asarray(ln_g, np.float32)
    ln_b = np.asarray(ln_b, np.float32)
    w_qkv = np.asarray(w_qkv, np.float32)
    w_out = np.asarray(w_out, np.float32)

    wg = w_qkv * ln_g[:, None]
    in_maps = []
    for c in range(8):
        b, g = c // 4, c % 4
        hs = slice(g * HPC * DH, (g + 1) * HPC * DH)        # inner dims of group
        wq = wg[:, 0 * DIM:1 * DIM][:, hs]                  # [1024, 256]
        wk = wg[:, 1 * DIM:2 * DIM][:, hs]
        wv = wg[:, 2 * DIM:3 * DIM][:, hs]
        wqk = np.concatenate([wq, wk], axis=1)              # [1024, 512]
        bq = ln_b @ w_qkv[:, 0 * DIM:1 * DIM][:, hs]
        bk = ln_b @ w_qkv[:, 1 * DIM:2 * DIM][:, hs]
        bv = (ln_b @ w_qkv[:, 2 * DIM:3 * DIM][:, hs]).reshape(1, -1)
        keep = (~attn_mask[b, g * HPC:(g + 1) * HPC]).transpose(0, 2, 1)
        in_maps.append({
            "x": np.ascontiguousarray(x[b]),
            "keep": np.ascontiguousarray(keep).astype(ml_dtypes.bfloat16),
            "wqk": np.ascontiguousarray(wqk),
            "wv": np.ascontiguousarray(wv),
            "wo": np.ascontiguousarray(w_out[hs, :]),
            "qkb": np.concatenate([bq, bk]).astype(np.float32),
            "vb": bv.astype(np.float32),
        })
    return in_maps


def kernel(x, attn_mask, ln_g, ln_b, w_qkv, w_out):
    nc = _get_program()
    in_maps = _shard_inputs(x, attn_mask, ln_g, ln_b, w_qkv, w_out)
    res = run_bass_kernel_spmd(nc, in_maps, list(range(8)))
    parts = [r["out"] for r in res.results]
    out = np.stack([parts[0] + parts[1] + parts[2] + parts[3],
                    parts[4] + parts[5] + parts[6] + parts[7]])
    return out.astype(np.float32)


# revision 24
# speedup vs baseline: 21237.9839x; 1.0740x over previous
"""Fused LayerNorm + multi-head self-attention + out-projection for TRN2,
sharded over 8 NeuronCores as (batch x head-group): core c -> batch c//4,
heads [4*(c%4), 4*(c%4)+4).

Per-core math (heads sharded, w_qkv column-sharded, w_out row-sharded):
  xn   = LayerNorm(x[b]) (ln_g folded into weights on host, ln_b via bias terms)
  qk_T = (w_qk.T @ xn_T)                  # [512, 2048]  (q rows 0:256, k rows 256:512)
  V    = xn @ w_v (+ ones col)            # [2048, 4*65] token-major, bf16
  per head h: S_T[k,q] = K_h @ Q_h.T ; P = exp(SCALE*S_T) * keep_T
              [O_h.T | rowsum] = [V_h|1].T.T @ P   (ones-col gives softmax denom)
  O_h.T /= rowsum (via reciprocal + ones-matmul broadcast)
  partial = O.T.T @ w_out[rows for these heads]    # [2048, 1024]
Host sums the 4 partials per batch. exp() needs no running-max: |SCALE*S| is
O(10) for unit-variance inputs, and masked entries are multiplied out after exp.
"""

import numpy as np
import ml_dtypes
from contextlib import ExitStack

import concourse.bass as bass
import concourse.tile as tile
from concourse import mybir
from concourse.masks import make_identity
from concourse.bass_utils import run_bass_kernel_spmd
import json as _json


def _split_waits(bir_json_bytes, max_waits=1):
    """This walrus build accepts only one sync-wait command per instruction;
    hoist extra Tile-emitted waits onto standalone EventSemaphore ops."""
    m = _json.loads(bir_json_bytes)
    n = 0
    for func in m["functions"]:
        for blk in func["blocks"]:
            out = []
            for inst in blk["instructions"]:
                si = inst.get("sync_info") or {}
                ow = si.get("on_wait") or []
                if len(ow) > max_waits:
                    for w in ow[:-max_waits]:
                        n += 1
                        out.append({
                            "engine": inst["engine"], "ins": [], "outs": [],
                            "name": f"WSPLIT-{n}",
                            "opcode": "EventSemaphore",
                            "sync_info": {"on_update": [], "on_wait": [w]},
                        })
                    si["on_wait"] = ow[-max_waits:]
                out.append(inst)
            blk["instructions"] = out
    return _json.dumps(m).encode()

F32 = mybir.dt.float32
F32R = mybir.dt.float32r


def _r(ap):
    return ap.bitcast(F32R)
BF16 = mybir.dt.bfloat16
AF = mybir.ActivationFunctionType

B, N, DIM = 2, 2048, 1024
HEADS, DH = 16, 64
HPC = 4                      # heads per core
SCALE = DH ** -0.5
LN_EPS = 1e-5
P = 128
NT = N // P                  # 16 token tiles
KD = DIM // P                # 8 contraction tiles over model dim
NEG = -30000.0               # additive mask value (unused; multiplicative used)


def build_program(ab=()):
    ab = set(ab)
    nc = bass.Bass()
    x_d = nc.dram_tensor("x", [N, DIM], F32, kind="ExternalInput")
    keep_d = nc.dram_tensor("keep", [HPC, N, N], BF16, kind="ExternalInput")
    wqk_d = nc.dram_tensor("wqk", [DIM, 2 * HPC * DH], F32, kind="ExternalInput")
    wv_d = nc.dram_tensor("wv", [DIM, HPC * DH], F32, kind="ExternalInput")
    wo_d = nc.dram_tensor("wo", [HPC * DH, DIM], F32, kind="ExternalInput")
    qkb_d = nc.dram_tensor("qkb", [2 * HPC * DH], F32, kind="ExternalInput")
    vb_d = nc.dram_tensor("vb", [1, HPC * DH], F32, kind="ExternalInput")
    out_d = nc.dram_tensor("out", [N, DIM], F32, kind="ExternalOutput")

    with tile.TileContext(nc) as tc, ExitStack() as ctx:
        persist = ctx.enter_context(tc.tile_pool(name="persist", bufs=1))

        ident = persist.tile([P, P], F32, tag="ident")
        make_identity(nc, ident)
        ones1f = persist.tile([1, P], F32, tag="ones1f")
        nc.vector.memset(ones1f, 1.0)
        ones1 = persist.tile([1, P], F32R, tag="ones1")
        nc.vector.tensor_copy(ones1, ones1f)
        eps_t = persist.tile([P, 1], F32, tag="eps")
        nc.vector.memset(eps_t, LN_EPS)
        zero_t = persist.tile([P, 1], F32, tag="zero")
        nc.vector.memset(zero_t, 0.0)

        # weights: DMA f32 staging then round-copy to f32r for the PE
        wqk_st = persist.tile([P, KD, 512], F32, tag="wqk_st")
        nc.sync.dma_start(out=wqk_st, in_=wqk_d.rearrange("(k p) c -> p k c", p=P))
        wqk_sb = persist.tile([P, KD, 512], F32R, tag="wqk")
        nc.vector.tensor_copy(wqk_sb, wqk_st)
        wv_st = persist.tile([P, KD, 256], F32, tag="wv_st")
        nc.sync.dma_start(out=wv_st, in_=wv_d.rearrange("(k p) c -> p k c", p=P))
        wv_sb = persist.tile([P, KD, 256], F32R, tag="wv")
        nc.vector.tensor_copy(wv_sb, wv_st)
        wo_st = persist.tile([P, 2, DIM], F32, tag="wo_st")
        nc.sync.dma_start(out=wo_st, in_=wo_d.rearrange("(k p) c -> p k c", p=P))
        wo_sb = persist.tile([P, 2, DIM], F32R, tag="wo")
        nc.vector.tensor_copy(wo_sb, wo_st)
        qkb_sb = persist.tile([P, 4], F32, tag="qkb")
        nc.sync.dma_start(out=qkb_sb, in_=qkb_d.rearrange("(t p) -> p t", p=P))
        vb_st = persist.tile([1, 256], F32, tag="vb_st")
        nc.sync.dma_start(out=vb_st, in_=vb_d[:, :])
        vb_sb = persist.tile([1, 256], F32R, tag="vb")
        nc.vector.tensor_copy(vb_sb, vb_st)

        # persistent activations
        qkT = persist.tile([P, 4, N], F32R, tag="qkT")
              # rows: [q01, q23, k01, k23]
        v_all = persist.tile([P, NT, HPC, DH + 1], BF16, tag="v_all")
        nc.gpsimd.memset(v_all[:, :, :, DH:DH + 1], 1.0)
        if "nov" in ab:
            nc.gpsimd.memset(v_all[:, :, :, 0:DH], 0.01)
        o_sb = persist.tile([P, 2, N], F32R, tag="o_sb")    # O_T rows: [h01, h23]

        # ---------------- Phase 1: LN + transpose + QKV/V matmuls -------------
        with tc.tile_pool(name="xnT_pool", bufs=1) as xnT_pool, \
             tc.tile_pool(name="xin", bufs=4) as xin_pool, \
             tc.tile_pool(name="stats", bufs=6) as st_pool, \
             tc.tile_pool(name="ps_a", bufs=4, space="PSUM") as ps_a, \
             tc.tile_pool(name="ps_qkv", bufs=2, space="PSUM") as ps_qkv, \
             tc.tile_pool(name="ps_v", bufs=2, space="PSUM") as ps_v:

            xnT = xnT_pool.tile([P, KD, N], F32R, tag="xnT")

            for tt in range(NT):
                xt = xin_pool.tile([P, DIM], F32, tag="x")
                nc.sync.dma_start(out=xt, in_=x_d[tt * P:(tt + 1) * P, :])
                # stats
                stats = st_pool.tile([P, 2, 6], F32, tag="bn")
                xt2 = xt.rearrange("p (s d) -> p s d", s=2)
                for s in range(2):
                    nc.vector.bn_stats(out=stats[:, s, :], in_=xt2[:, s, :])
                mv = st_pool.tile([P, 2], F32, tag="mv")
                nc.vector.bn_aggr(out=mv, in_=stats)
                std = st_pool.tile([P, 1], F32, tag="std")
                nc.scalar.activation(std, mv[:, 1:2], AF.Sqrt, bias=eps_t)
                rstd = st_pool.tile([P, 1], F32, tag="rstd")
                nc.vector.reciprocal(rstd, std)
                nmr = st_pool.tile([P, 1], F32, tag="nmr")
                nc.vector.tensor_mul(nmr, mv[:, 0:1], rstd)
                nc.vector.tensor_scalar_mul(nmr, nmr, -1.0)
                # xn = rstd*x - mean*rstd   (in place)
                nc.vector.tensor_scalar(xt, xt, rstd, nmr,
                                        op0=mybir.AluOpType.mult,
                                        op1=mybir.AluOpType.add)
                # transpose 8 [128,128] blocks -> xnT[:, k, tt*128:...]
                if "notrans" in ab:
                    if tt == 0:
                        nc.gpsimd.memset(xnT, 0.5)
                else:
                    for k in range(KD):
                        tp = ps_a.tile([P, P], F32, tag="tp")
                        nc.tensor.transpose(tp, xt[:, k * P:(k + 1) * P], ident)
                        if k % 2:
                            nc.scalar.copy(xnT[:, k, tt * P:(tt + 1) * P], tp)
                        else:
                            nc.vector.tensor_copy(xnT[:, k, tt * P:(tt + 1) * P], tp)

            # QKV (transposed): psum[cols 128, tok 512] += wqk_tile.T @ xnT
            if "noqkv" in ab:
                nc.gpsimd.memset(qkT, 0.01)
            for m in range([] and 4 if False else (0 if "noqkv" in ab else 4)):
                for tb in range(4):
                    pq = ps_qkv.tile([P, 512], F32, tag="pq")
                    for k in range(KD):
                        nc.tensor.matmul(
                            pq, wqk_sb[:, k, m * P:(m + 1) * P],
                            xnT[:, k, tb * 512:(tb + 1) * 512],
                            start=(k == 0), stop=(k == KD - 1))
                    if tb % 2:
                        nc.scalar.activation(qkT[:, m, tb * 512:(tb + 1) * 512], pq,
                                             AF.Identity, bias=qkb_sb[:, m:m + 1])
                    else:
                        nc.vector.tensor_scalar_add(
                            qkT[:, m, tb * 512:(tb + 1) * 512], pq, qkb_sb[:, m:m + 1])

            # V token-major: psum[tok 128, 256] = ones.T@vb + xnT_tile.T @ wv
            for tt in range(0 if "nov" in ab else NT):
                pv = ps_v.tile([P, 256], F32, tag="pv")
                if "novb" not in ab:
                    nc.tensor.matmul(pv, ones1, vb_sb, start=True, stop=False)
                for k in range(KD):
                    nc.tensor.matmul(
                        pv, xnT[:, k, tt * P:(tt + 1) * P], wv_sb[:, k, :],
                        start=("novb" in ab and k == 0), stop=(k == KD - 1))
                if tt % 2:
                    nc.scalar.copy(v_all[:, tt, :, 0:DH],
                                   pv.rearrange("p (h d) -> p h d", h=HPC))
                else:
                    nc.vector.tensor_copy(
                        v_all[:, tt, :, 0:DH],
                        pv.rearrange("p (h d) -> p h d", h=HPC))

        # ---------------- Phase 2: attention per head -------------------------
        with tc.tile_pool(name="keep", bufs=5) as keep_pool, \
             tc.tile_pool(name="pexp", bufs=6) as p_pool, \
             tc.tile_pool(name="rec", bufs=4) as rec_pool, \
             tc.tile_pool(name="bcs", bufs=2) as bcs_pool, \
             tc.tile_pool(name="ps_s", bufs=2, space="PSUM") as ps_s, \
             tc.tile_pool(name="ps_o", bufs=2, space="PSUM") as ps_o:

            for h in ([] if "noattn" in ab else range(HPC)):
                qrow = (h % 2) * DH
                qm, km = h // 2, 2 + h // 2
                for qb in range(2):
                    cs = slice(qb * 1024, (qb + 1) * 1024)
                    o_ps = ps_o.tile([DH + 1, 1024], F32, tag="o")
                    for kt in range(NT):
                        kp = keep_pool.tile([P, 1024], BF16, tag="keep")
                        if "nokeepdma" in ab:
                            nc.gpsimd.memset(kp, 1.0)
                        else:
                            nc.sync.dma_start(
                                out=kp, in_=keep_d[h, kt * P:(kt + 1) * P, cs])
                        sp = ps_s.tile([P, 1024], F32, tag="s")
                        for j in range(2):
                            qs = qb * 1024 + j * 512
                            nc.tensor.matmul(
                                sp[:, j * 512:(j + 1) * 512],
                                qkT[qrow:qrow + DH, km, kt * P:(kt + 1) * P],
                                qkT[qrow:qrow + DH, qm, qs:qs + 512],
                                start=True, stop=True)
                        pe = p_pool.tile([P, 1024], BF16, tag="p")
                        if "expdve" in ab:
                            nc.vector.tensor_copy(pe, sp)
                        else:
                            nc.scalar.activation(pe, sp, AF.Exp, bias=zero_t, scale=SCALE)
                        if "nomult" not in ab:
                            eng = nc.gpsimd if ("gpsmult" in ab and kt % 2) else nc.vector
                            eng.tensor_mul(pe, pe, kp)
                        for j in range(2):
                            nc.tensor.matmul(
                                o_ps[:, j * 512:(j + 1) * 512],
                                v_all[:, kt, h, :],
                                pe[:, j * 512:(j + 1) * 512],
                                start=(kt == 0), stop=(kt == NT - 1))
                # normalize + evict into o_sb
                    orow = (h % 2) * DH
                    om = h // 2
                    rec = rec_pool.tile([1, 1024], F32R, tag="rec")
                    with nc.allow_low_precision(reason="f32r rounding for PE broadcast"):
                        nc.vector.reciprocal(rec, o_ps[DH:DH + 1, :])
                    bc = ps_s.tile([DH, 1024], F32, tag="s")
                    for j in range(2):
                        nc.tensor.matmul(
                            bc[:, j * 512:(j + 1) * 512], ones1[:, 0:DH],
                            rec[:, j * 512:(j + 1) * 512], start=True, stop=True)
                    bcs = bcs_pool.tile([DH, 1024], F32, tag="bcs")
                    nc.vector.tensor_copy(bcs, bc)
                    nc.vector.tensor_mul(
                        o_sb[orow:orow + DH, om, cs], o_ps[0:DH, :], bcs)

        # ---------------- Phase 3: out projection -----------------------------
        with tc.tile_pool(name="oev", bufs=4) as oev_pool, \
             tc.tile_pool(name="ps_out", bufs=2, space="PSUM") as ps_out:
            for tt in range(NT):
                po = ps_out.tile([P, DIM], F32, tag="po")
                for nn2 in range(2):
                    for k in range(2):
                        nc.tensor.matmul(
                            po[:, nn2 * 512:(nn2 + 1) * 512],
                            o_sb[:, k, tt * P:(tt + 1) * P],
                            wo_sb[:, k, nn2 * 512:(nn2 + 1) * 512],
                            start=(k == 0), stop=(k == 1))
                ot = oev_pool.tile([P, DIM], F32, tag="ot")
                if tt % 2:
                    nc.scalar.copy(ot, po)
                else:
                    nc.vector.tensor_copy(ot, po)
                nc.sync.dma_start(out=out_d[tt * P:(tt + 1) * P, :], in_=ot)

    return nc


_NC_CACHE = {}


def _get_program():
    if "nc" not in _NC_CACHE:
        nc = build_program()
        data = _split_waits(nc.to_json_bytes())
        nc.to_json_bytes = lambda: data
        _NC_CACHE["nc"] = nc
    return _NC_CACHE["nc"]


def _shard_inputs(x, attn_mask, ln_g, ln_b, w_qkv, w_out):
    x = np.asarray(x, np.float32)
    attn_mask = np.asarray(attn_mask)
    ln_g = np.asarray(ln_g, np.float32)
    ln_b = np.asarray(ln_b, np.float32)
    w_qkv = np.asarray(w_qkv, np.float32)
    w_out = np.asarray(w_out, np.float32)

    wg = w_qkv * ln_g[:, None]
    in_maps = []
    for c in range(8):
        b, g = c // 4, c % 4
        hs = slice(g * HPC * DH, (g + 1) * HPC * DH)        # inner dims of group
        wq = wg[:, 0 * DIM:1 * DIM][:, hs]                  # [1024, 256]
        wk = wg[:, 1 * DIM:2 * DIM][:, hs]
        wv = wg[:, 2 * DIM:3 * DIM][:, hs]
        wqk = np.concatenate([wq, wk], axis=1)              # [1024, 512]
        bq = ln_b @ w_qkv[:, 0 * DIM:1 * DIM][:, hs]
        bk = ln_b @ w_qkv[:, 1 * DIM:2 * DIM][:, hs]
        bv = (ln_b @ w_qkv[:, 2 * DIM:3 * DIM][:, hs]).reshape(1, -1)
        keep = (~attn_mask[b, g * HPC:(g + 1) * HPC]).transpose(0, 2, 1)
        in_maps.append({
            "x": np.ascontiguousarray(x[b]),
            "keep": np.ascontiguousarray(keep).astype(ml_dtypes.bfloat16),
            "wqk": np.ascontiguousarray(wqk),
            "wv": np.ascontiguousarray(wv),
            "wo": np.ascontiguousarray(w_out[hs, :]),
            "qkb": np.concatenate([bq, bk]).astype(np.float32),
            "vb": bv.astype(np.float32),
        })
    return in_maps


def kernel(x, attn_mask, ln_g, ln_b, w_qkv, w_out):
    nc = _get_program()
    in_maps = _shard_inputs(x, attn_mask, ln_g, ln_b, w_qkv, w_out)
    res = run_bass_kernel_spmd(nc, in_maps, list(range(8)))
    parts = [r["out"] for r in res.results]
    out = np.stack([parts[0] + parts[1] + parts[2] + parts[3],
                    parts[4] + parts[5] + parts[6] + parts[7]])
    return out.astype(np.float32)
